# revision 51
# baseline (speedup 1.0000x reference)
"""EPLL MoE-routing kernel for 8 trn2 NeuronCores — fp8 DoubleRow, transposed.

Device (per core, per beta): routing matmul producing lp[p, k]
    lp[p, k] = sum_r OTaug[r, p] * Aaug[r, k]
with r = 702 rows (666 sym-packed outer products of centered patches +
36 linear rows; constant term added on host), K = 200 moving cols,
patches sharded 8 ways (7936/core, 62 groups of 128).
Contraction packed as 3 chunks of [h, 2 DoubleRow rows] with
h = 128/128/96 (LdWeights requires 32-aligned partition heights).
Patch-product chunks are the STATIONARY operand ([h, 2, 128] per
128-patch group), the A operand is MOVING ([h, 2, 200]) — each
DoubleRow matmul costs 100 PE cycles; psum output [128p, 200k] has
patches on partitions.  fp8 e4m3 operands, fp32 PSUM accumulate.

Schedule: inputs streamed just-in-time on SP + Pool queues (Act
issues a single early DMA, keeping its sequencer free for evictions
— DMA issues block the issuing engine's in-order SEQ); the Act
activation table is pre-warmed at t=0 by a dummy copy; PSUM
evictions to fp8 alternate DVE/Act (the only engines that can read
PSUM) with small ping-pong units at the tail; outputs flushed in six
plain DMAs (pool early, SP late, the final one small).

Self-contained: shapes hardcoded for y[1,1,256,256], K=200, D=36.
"""

import sys

sys.path.insert(0, "/opt/trn_rl_repo")

import numpy as np
import ml_dtypes

B, C, H, W = 1, 1, 256, 256
PS = 6
K = 200
KPAD = 200                 # moving cols (psum out free = PE cost)
D = PS * PS * C            # 36
SIGMA_SQ = 0.01
BETAS = [b / SIGMA_SQ for b in (1.0, 4.0, 8.0, 16.0, 32.0)]
NPIX = C * H * W

NI = H - PS + 1            # 251
P = NI * NI                # 63001
N_CORES = 8
PPC = 7936                 # padded patches per core (8*7936 = 63488)
NPG = PPC // 128           # 62 patch groups of 128

NSYM = D * (D + 1) // 2    # 666
NROW = NSYM + D            # 702
CH = [128, 128, 96]        # contraction chunk heights (x2 rows each)
CBASE = [0, 256, 512]

CENTER = 0.5
SQ = 2.0                   # product-row scale
SL = 2.0                   # linear-row scale
OSCALE = 16.0              # host multiplies fp8 output by this

E4 = ml_dtypes.float8_e4m3fn

_IU, _IV = np.triu_indices(D)
_SYM_SCALE = np.where(_IU == _IV, 1.0, 2.0).astype(np.float32)


def _patch_linear_indices():
    i0 = np.arange(NI)
    rows = i0[:, None, None, None] + np.arange(PS)[None, None, :, None]
    cols = i0[None, :, None, None] + np.arange(PS)[None, None, None, :]
    return (rows * W + cols).reshape(NI * NI, PS * PS).astype(np.int64)


LIN = _patch_linear_indices()          # [P, D]

_STATE = {}


# ---- schedule ----------------------------------------------------------
# input DMAs after the merged A+g0 one, in global group order:
# (ngroups, queue).  dma_start calls are emitted in this order.
IN_PLAN = [(2, "pool"), (2, "sp"), (3, "pool"), (3, "sp"), (4, "pool"),
           (4, "sp"), (5, "pool"), (9, "act"), (5, "sp"), (7, "pool"),
           (6, "sp"), (5, "pool"), (6, "sp")]
# units: (mode, ngroups, arg); mode "ev": arg = list of (engine, ng);
# mode "dir": arg = queue for the direct fp32 psum DMA
UNITS = [("ev", 4, [("dve", 4)]), ("ev", 4, [("act", 4)]),
         ("ev", 4, [("dve", 4)]), ("ev", 4, [("act", 4)]),
         ("ev", 4, [("dve", 4)]), ("ev", 4, [("act", 4)]),
         ("ev", 4, [("dve", 4)]), ("ev", 4, [("act", 4)]),
         ("ev", 4, [("dve", 4)]), ("ev", 4, [("act", 4)]),
         ("ev", 4, [("dve", 4)]), ("ev", 4, [("act", 4)]),
         ("ev", 4, [("dve", 4)]), ("ev", 4, [("act", 4)]),
         ("ev", 2, [("dve", 2)]), ("ev", 2, [("act", 2)]),
         ("ev", 2, [("dve", 2)])]
# fp8 flushes: (first_unit, last_unit, queue) — contiguous "ev" units
FLUSHES = [(0, 2, "pool"), (3, 5, "pool"), (6, 8, "sp"),
           (9, 11, "pool"), (12, 13, "sp"), (14, 16, "sp")]


def _build_bass():
    from concourse import bacc, mybir
    from concourse.tile import TileContext

    nc = bacc.Bacc("TRN2", target_bir_lowering=False, debug=False,
                   num_devices=N_CORES)

    assert 1 + sum(n for n, _ in IN_PLAN) == NPG
    unit_ng = [n for _, n, _ in UNITS]
    assert sum(unit_ng) == NPG
    in_cols = [128] + [128 * n for n, _ in IN_PLAN]
    in_edges = np.cumsum([0] + in_cols).tolist()
    unit_edges = np.cumsum([0] + unit_ng).tolist()

    a_dram = nc.dram_tensor("a", [128, 3, 2, KPAD], mybir.dt.float8e4,
                            kind="ExternalInput")
    ot_drams = [
        nc.dram_tensor(f"ot{g}", [128, 3, 2, 128 * ng], mybir.dt.float8e4,
                       kind="ExternalInput")
        for g, (ng, _) in enumerate(IN_PLAN)]
    ot0_dram = nc.dram_tensor("ot_first", [128, 3, 2, 128],
                              mybir.dt.float8e4, kind="ExternalInput")
    lp_dram = nc.dram_tensor("lp", [128, NPG, K], mybir.dt.float8e4,
                             kind="ExternalOutput")
    lpd_drams = {}
    for u, (mode, ng, arg) in enumerate(UNITS):
        if mode == "dir":
            lpd_drams[u] = nc.dram_tensor(
                f"lpd{u}", [128, ng, K], mybir.dt.float32,
                kind="ExternalOutput")

    DR = mybir.MatmulPerfMode.DoubleRow

    with TileContext(nc) as tc:
        qmap = {"sp": nc.sync, "act": nc.scalar, "pool": nc.gpsimd}
        emap = {"act": nc.scalar, "dve": nc.vector}
        with (
            tc.tile_pool(name="apool", bufs=1) as apool,
            tc.tile_pool(name="otpool", bufs=1) as otpool,
            tc.tile_pool(name="lppool", bufs=1) as lppool,
            tc.tile_pool(name="psum", bufs=4, space="PSUM") as pspool,
        ):
            # warm the Act activation table before evictions start
            warm0 = apool.tile([128, 8], mybir.dt.float32, tag="warm0")
            warm1 = apool.tile([128, 8], mybir.dt.float32, tag="warm1")
            nc.vector.memset(warm0[:], 0.0)
            nc.scalar.copy(warm1[:], warm0[:])

            a_sb = apool.tile([128, 3, 2, KPAD], mybir.dt.float8e4)
            nc.gpsimd.dma_start(a_sb[:], a_dram.ap())
            ot0_sb = apool.tile([128, 3, 2, 128], mybir.dt.float8e4,
                                tag="ot_first")
            nc.sync.dma_start(ot0_sb[:], ot0_dram.ap())

            # (tile, first group col, last group col, col offset in tile)
            in_tiles = [(ot0_sb, 0, 128, 0)]
            for g, (ng, q) in enumerate(IN_PLAN):
                gcols = 128 * ng
                ot = otpool.tile([128, 3, 2, gcols], mybir.dt.float8e4,
                                 tag=f"ot{g}")
                qmap[q].dma_start(ot[:], ot_drams[g].ap())
                in_tiles.append((ot, in_edges[g + 1], in_edges[g + 2], 0))

            lp_tiles = {}
            for fi, (ua, ub, q) in enumerate(FLUSHES):
                ng = unit_edges[ub + 1] - unit_edges[ua]
                t = lppool.tile([128, ng, K], mybir.dt.float8e4,
                                tag=f"lp{fi}")
                for u in range(ua, ub + 1):
                    lp_tiles[u] = (t, unit_edges[ua])

            for u, (mode, ng, arg) in enumerate(UNITS):
                pg = unit_edges[u]
                ps = pspool.tile([128, 4, 256], mybir.dt.float32, tag="ps")
                for j in range(ng):
                    c0 = (pg + j) * 128
                    for ot, g0, g1, coff in in_tiles:
                        if g0 <= c0 < g1:
                            break
                    off = coff + (c0 - g0)
                    for c in range(3):
                        h = CH[c]
                        nc.tensor.matmul(
                            ps[:, j, 0:KPAD],
                            ot[0:h, c, :, off:off + 128],
                            a_sb[0:h, c],
                            start=(c == 0), stop=(c == 2),
                            perf_mode=DR)
                if mode == "ev":
                    t, base = lp_tiles[u]
                    lo = pg - base
                    j0 = 0
                    for eng_name, sng in arg:
                        dst = t[:, lo + j0:lo + j0 + sng, :]
                        src = ps[:, j0:j0 + sng, 0:K]
                        if eng_name == "act":
                            emap[eng_name].copy(dst, src)
                        else:
                            emap[eng_name].tensor_copy(dst, src)
                        j0 += sng
                    for ua, ub, q in FLUSHES:
                        if ub == u:
                            qmap[q].dma_start(
                                lp_dram.ap()[:, unit_edges[ua]:
                                             unit_edges[ub + 1], :],
                                t[:])
                else:
                    qmap[arg].dma_start(lpd_drams[u].ap(),
                                        ps[:, 0:ng, 0:K])
    nc.finalize()
    return nc


def _get_state():
    if not _STATE:
        _STATE["nc"] = _build_bass()
    return _STATE


def _pack_rows(rows_e4):
    """rows_e4: [NROW(702), cols] fp8 -> [128, 3, 2, cols] chunk layout."""
    out = np.zeros((128, 3, 2, rows_e4.shape[1]), E4)
    for c in range(3):
        for i in range(2):
            lo = CBASE[c] + i * CH[c]
            hi = min(lo + CH[c], NROW)
            if lo >= NROW:
                continue
            out[0:hi - lo, c, i] = rows_e4[lo:hi]
    return out


def kernel(y, mu, log_weights, eigvecs, eigvals):
    from concourse import bass_utils

    y = np.asarray(y, np.float32)
    mu = np.asarray(mu, np.float32)
    lw = np.asarray(log_weights, np.float32)
    U = np.asarray(eigvecs, np.float32)
    ev = np.asarray(eigvals, np.float32)

    st = _get_state()
    nc = st["nc"]

    yf = y.reshape(-1)
    x = yf.copy()

    mult = np.bincount(LIN.ravel(), minlength=NPIX).astype(np.float32)
    inv_mult = 1.0 / mult

    in_cols = [128] + [128 * n for n, _ in IN_PLAN]
    in_edges = np.cumsum([0] + in_cols).tolist()
    unit_ng = [n for _, n, _ in UNITS]
    unit_edges = np.cumsum([0] + unit_ng).tolist()

    for beta in BETAS:
        reg = 1.0 / beta
        l = ev + reg                                        # [K, D]
        il = (1.0 / l).astype(np.float32)
        A = np.einsum("kde,ke,kfe->kdf", U, il, U)          # [K, D, D]
        E = np.einsum("kde,ke,kfe->kdf", U, ev * il, U)     # [K, D, D]
        logdet = np.log(l).sum(1)
        mu_c = mu - CENTER
        Amu = np.einsum("kdf,kf->kd", A, mu_c)              # [K, D]
        muAmu = np.einsum("kd,kd->k", mu_c, Amu)
        cterm = (lw - 0.5 * logdet - 0.5 * muAmu).astype(np.float32)

        # moving operand [NROW, KPAD] fp8
        Arows = np.zeros((NROW, KPAD), np.float32)
        Arows[:NSYM, :K] = (-0.5 / (SQ * OSCALE) * _SYM_SCALE[:, None]
                            * A[:, _IU, _IV].T)
        Arows[NSYM:, :K] = Amu.T / (SL * OSCALE)
        a_pack = _pack_rows(Arows.astype(E4))               # [128,3,2,200]

        # stationary operand rows [NROW, P] fp8, shard per core
        pat = x[LIN]                                        # [P, D] f32
        pc = pat - CENTER
        rows = np.empty((P, NROW), np.float32)
        np.multiply(pc[:, _IU], pc[:, _IV], out=rows[:, :NSYM])
        rows[:, :NSYM] *= SQ
        rows[:, NSYM:] = pc * SL
        rows_e4 = rows.astype(E4)                           # [P, NROW]

        in_maps = []
        for cidx in range(N_CORES):
            p0 = cidx * PPC
            p1 = min(p0 + PPC, P)
            slab = np.zeros((NROW, PPC), E4)
            slab[:, 0:p1 - p0] = rows_e4[p0:p1].T
            packed = _pack_rows(slab)                       # [128,3,2,PPC]
            m = {"a": a_pack,
                 "ot_first": np.ascontiguousarray(packed[:, :, :, 0:128])}
            for g in range(len(IN_PLAN)):
                m[f"ot{g}"] = np.ascontiguousarray(
                    packed[:, :, :, in_edges[g + 1]:in_edges[g + 2]])
            in_maps.append(m)

        res = bass_utils.run_bass_kernel_spmd(
            nc, in_maps, core_ids=list(range(N_CORES)))

        # assemble lp [PPC, K] per core from fp8 + direct fp32 outputs
        lp_all = []
        for r in res.results:
            lp_core = np.array(r["lp"]).astype(np.float32)  # [128, NPG, K]
            for u, (mode, ng, arg) in enumerate(UNITS):
                if mode == "dir":
                    g0 = unit_edges[u]
                    lp_core[:, g0:g0 + ng, :] = r[f"lpd{u}"]
            lp_all.append(lp_core.transpose(1, 0, 2).reshape(PPC, K))
        lp_all = np.concatenate(lp_all, axis=0)             # [8*PPC, K]
        lp_full = np.concatenate(
            [lp_all[cidx * PPC: cidx * PPC + min(PPC, P - cidx * PPC)]
             for cidx in range(N_CORES)], axis=0)           # [P, K]
        lp_full = lp_full * OSCALE + cterm[None, :]

        # exact top-candidate repair
        TOPC = 4
        cand = np.argpartition(-lp_full, TOPC, axis=1)[:, :TOPC + 1]
        best_v = None
        best_k = None
        for r in range(TOPC + 1):
            kr = cand[:, r]
            quad = np.einsum("pde,pd,pe->p", A[kr], pc, pc, optimize=True)
            lin = np.einsum("pd,pd->p", Amu[kr], pc)
            v = -0.5 * quad + lin + cterm[kr]
            if best_v is None:
                best_v, best_k = v, kr.copy()
            else:
                m2 = v > best_v
                best_v = np.where(m2, v, best_v)
                best_k = np.where(m2, kr, best_k)
        ks = best_k

        est = np.einsum("pde,pe->pd", E[ks], pat)
        xt = np.bincount(LIN.ravel(), weights=est.ravel().astype(np.float64),
                        minlength=NPIX).astype(np.float32)
        xt *= inv_mult
        cdf = beta * SIGMA_SQ
        x = (yf + cdf * xt) / (1.0 + cdf)

    return x.reshape(B, C, H, W).astype(np.float32)
